# revision 12
# baseline (speedup 1.0000x reference)
"""Trainium2 distributed kernel for the multi-query sparse-attention block.

Sharding: 8 cores = 2 batches x 4 head-groups (4 heads each).
J (key/value axis) is host-permuted to [self(2048) | ctx(256) | null(1) | pad(127)]
and the attention bias arrives pre-transposed (j-major), mask-folded and
pre-exponentiated in bf16:  attn_weight = exp(q.k) * expb.

Attention processes HEAD PAIRS: head A lives on SBUF partitions 0-63, head B
on 64-127 (kT/qT duplicated/stacked), so the two qk matmuls run CONCURRENTLY
on the PE's two 64-row tiles (tile_position (0,0) / (64,0)).  The two sim
tiles land in adjacent PSUM banks and are exponentiated by a single 1024-wide
scalar-engine instruction (the scalar engine exp stream is the pacing
resource).  Softmax runs without max-subtraction; denominators come from a
ones-column appended to V and are reciprocated on the vector engine.  All
layernorm rstds use exp(-0.5*ln(var+eps)) so the scalar engine never switches
activation tables.  Output projection partials are ReduceScattered over each
4-core batch group; final layernorms are pipelined into the attention loop.
"""

import sys

sys.path.insert(0, "/opt/trn_rl_repo")

import numpy as np
import ml_dtypes

import concourse.bass as bass
import concourse.mybir as mybir
import concourse.tile as tile
from concourse import bacc
from concourse.bass_utils import run_bass_kernel_spmd
from concourse.masks import make_identity

F32 = mybir.dt.float32
F32R = mybir.dt.float32r
BF16 = mybir.dt.bfloat16
AF = mybir.ActivationFunctionType
ALU = mybir.AluOpType

B, N, D = 2, 2048, 1024
H, DH = 16, 64
C, CD = 256, 512
J = C + 1 + N          # 2305
JP = 19 * 128          # 2432 padded
HPC = 4                # heads per core
EPS = 1e-5

_cache = {}


def _ln_stats(nc, pool, srcs, d, eps_ap, p=128):
    """Batched LN stats for k [p, d] tiles.  rstd = exp(-0.5*ln(var+eps))
    (stays on the ln/exp activation table - no table switch).
    Returns (rstd_aps, mean_aps, negmr_aps)."""
    k = len(srcs)
    ns = d // 512
    mv = pool.tile([p, k, 2], F32, tag="lnmv")
    for j, s_ap in enumerate(srcs):
        stats = pool.tile([p, ns, 6], F32, tag="lnst")
        r = s_ap.rearrange("p (n f) -> p n f", f=512)
        for s in range(ns):
            nc.vector.bn_stats(out=stats[:, s, :], in_=r[:, s, :])
        nc.vector.bn_aggr(out=mv[:, j, :], in_=stats[:, :, :])
    mvr = mv[:].rearrange("p k two -> p (k two)")
    lnv = pool.tile([p, k], F32, tag="lnlv")
    nc.scalar.activation(lnv[:], mvr[:, 1::2], AF.Ln, bias=eps_ap[0:p, :])
    rstd = pool.tile([p, k], F32, tag="lnrs")
    nc.scalar.activation(rstd[:], lnv[:], AF.Exp, scale=-0.5)
    negmr = pool.tile([p, k], F32, tag="lnnm")
    nc.vector.scalar_tensor_tensor(
        out=negmr[:], in0=mvr[:, 0::2], scalar=-1.0, in1=rstd[:],
        op0=ALU.mult, op1=ALU.mult)
    return ([rstd[:, j:j + 1] for j in range(k)],
            [mvr[:, 2 * j:2 * j + 1] for j in range(k)],
            [negmr[:, j:j + 1] for j in range(k)])


def build():
    nc = bacc.Bacc("TRN2", target_bir_lowering=False, debug=False, num_devices=8)

    expb = nc.declare_dram_parameter("expb", [HPC, JP, N], BF16, isOutput=False)
    x_in = nc.declare_dram_parameter("x", [N, D], F32, isOutput=False)
    ctx_in = nc.declare_dram_parameter("ctxt", [C, CD], F32, isOutput=False)
    nullk = nc.declare_dram_parameter("nullk", [128, 1], F32, isOutput=False)
    nullv = nc.declare_dram_parameter("nullv", [1, DH], F32, isOutput=False)
    wq_in = nc.declare_dram_parameter("wq", [D, 256], F32, isOutput=False)
    wkv_in = nc.declare_dram_parameter("wkv", [D, 128], F32, isOutput=False)
    wctx_in = nc.declare_dram_parameter("wctx", [CD, 128], F32, isOutput=False)
    bctx_in = nc.declare_dram_parameter("bctx2", [1, 128], F32, isOutput=False)
    wout_in = nc.declare_dram_parameter("wout", [256, D], F32, isOutput=False)
    outg_in = nc.declare_dram_parameter("outg", [1, D], F32, isOutput=False)
    out_ext = nc.declare_dram_parameter("out", [N // 4, D], F32, isOutput=True)

    rs_in = [nc.dram_tensor(f"rs_in{c}", [512, D], BF16) for c in range(3)]
    rs_out = [nc.dram_tensor(f"rs_out{c}", [128, D], BF16) for c in range(3)]
    # last chunk split in two for a shorter exposed tail
    rs_in3 = [nc.dram_tensor(f"rs_in3{c}", [256, D], BF16) for c in range(2)]
    rs_out3 = [nc.dram_tensor(f"rs_out3{c}", [64, D], BF16) for c in range(2)]

    with tile.TileContext(nc) as tc:
        with tc.tile_pool(name="persist", bufs=1) as pp:
            wq_r = pp.tile([128, 8, 256], BF16)
            wkv_r = pp.tile([128, 8, 128], BF16)
            wctx_r = pp.tile([128, 4, 128], BF16)
            wout_r = pp.tile([128, 2, 1024], F32R)
            bctx_r = pp.tile([1, 128], BF16)
            ones_r = pp.tile([1, 1024], BF16)
            ident0 = pp.tile([128, 128], F32)
            ident_r = pp.tile([128, 128], BF16)
            gamma_bc = pp.tile([128, 1024], F32)
            qT2 = pp.tile([128, 2, N], BF16)     # [pairstack, m, n]
            kT = pp.tile([128, JP], BF16)        # both halves hold same kT
            vext = pp.tile([128, 19 * 65], BF16)
            aoT0 = pp.tile([128, N], F32R)
            aoT1 = pp.tile([128, N], F32R)
            aoT = [aoT0, aoT1]

            nc.gpsimd.dma_start(out=wq_r[:], in_=wq_in.rearrange("(c p) f -> p c f", p=128))
            nc.gpsimd.dma_start(out=wkv_r[:], in_=wkv_in.rearrange("(c p) f -> p c f", p=128))
            nc.gpsimd.dma_start(out=wctx_r[:], in_=wctx_in.rearrange("(c p) f -> p c f", p=128))
            nc.gpsimd.dma_start(out=wout_r[:], in_=wout_in.rearrange("(c p) f -> p c f", p=128))
            nc.gpsimd.dma_start(out=bctx_r[:], in_=bctx_in[:])
            nc.gpsimd.dma_start(out=kT[:, 2304:2305], in_=nullk[:])

            eps_t = pp.tile([128, 1], F32)
            nc.gpsimd.memset(eps_t[:], EPS)
            nc.vector.memset(kT[:, 2305:2432], 0.0)
            nc.vector.memset(vext[:, 18 * 65:18 * 65 + 64], 0.0)
            nc.gpsimd.dma_start(out=vext[0:1, 18 * 65:18 * 65 + 64], in_=nullv[:])
            for jb in range(19):
                nc.vector.memset(vext[:, jb * 65 + 64:jb * 65 + 65], 1.0)

            o1 = pp.tile([1, 1024], F32)
            nc.vector.memset(o1[:], 1.0)
            nc.scalar.copy(ones_r[:], o1[:])
            make_identity(nc, ident0[:])
            nc.scalar.copy(ident_r[:], ident0[:])

            og_sb = pp.tile([1, 1024], F32)
            nc.sync.dma_start(out=og_sb[:], in_=outg_in[:])
            nc.gpsimd.partition_broadcast(gamma_bc[:], og_sb[:])

            # ---------------- context tokens -> kT/vext ----------------
            # (emitted first: its compute overlaps the large x DMAs)
            with tc.tile_pool(name="cwork", bufs=2) as cw, \
                 tc.tile_pool(name="cstat", bufs=2) as cs, \
                 tc.tile_pool(name="cps", bufs=2, space="PSUM") as cps:
                cnT = pp.tile([128, 4, 256], BF16)
                cts = []
                for t in range(2):
                    ct = cw.tile([128, CD], F32, tag="ct")
                    nc.sync.dma_start(out=ct[:], in_=ctx_in[t * 128:(t + 1) * 128, :])
                    cts.append(ct)
                rstds, means, negmrs = _ln_stats(
                    nc, cs, [c[:] for c in cts], CD, eps_t)
                for t in range(2):
                    cn = cw.tile([128, CD], BF16, tag="cn")
                    nc.scalar.activation(cn[:], cts[t][:], AF.Identity,
                                         bias=negmrs[t], scale=rstds[t])
                    for c in range(4):
                        pt = cps.tile([128, 128], BF16, tag="ctp")
                        nc.tensor.matmul(pt[:], cn[:, c * 128:(c + 1) * 128],
                                         ident_r[:], is_transpose=True,
                                         start=True, stop=True)
                        nc.scalar.copy(cnT[:, c, t * 128:(t + 1) * 128],
                                       pt[:])
                pck = cps.tile([64, 256], F32, tag="ck")
                for c in range(4):
                    nc.tensor.matmul(pck[:], wctx_r[:, c, 0:64], cnT[:, c, :],
                                     start=(c == 0), stop=False)
                nc.tensor.matmul(pck[:], bctx_r[:, 0:64], ones_r[:, 0:256],
                                 start=False, stop=True)
                nc.scalar.copy(kT[0:64, 2048:2304], pck[:])
                nc.scalar.copy(kT[64:128, 2048:2304], pck[:])
                for t in range(2):
                    pcv = cps.tile([128, 64], F32, tag="cv")
                    for c in range(4):
                        nc.tensor.matmul(pcv[:], cnT[:, c, t * 128:(t + 1) * 128],
                                         wctx_r[:, c, 64:128],
                                         start=(c == 0), stop=False)
                    nc.tensor.matmul(pcv[:], ones_r[:, 0:128], bctx_r[:, 64:128],
                                     start=False, stop=True)
                    nc.vector.tensor_copy(vext[:, (16 + t) * 65:(16 + t) * 65 + 64],
                                          pcv[:])

            # ---------------- x: LN + transpose + projections ----------------
            with tc.tile_pool(name="xt", bufs=16) as xp, \
                 tc.tile_pool(name="xst", bufs=2) as xs, \
                 tc.tile_pool(name="xnt", bufs=2) as xnp, \
                 tc.tile_pool(name="vtmp", bufs=2) as vtp, \
                 tc.tile_pool(name="xps", bufs=3, space="PSUM") as xps, \
                 tc.tile_pool(name="pps", bufs=2, space="PSUM") as pps:
                for ic in range(4):
                    xnT = xnp.tile([128, 8, 512], BF16, tag="xnT")
                    xts = []
                    for tb in range(4):
                        i0 = ic * 512 + tb * 128
                        xt = xp.tile([128, D], F32, tag="xt")
                        nc.sync.dma_start(out=xt[:], in_=x_in[i0:i0 + 128, :])
                        xts.append(xt)
                    rstds, means, negmrs = _ln_stats(
                        nc, xs, [t[:] for t in xts], D, eps_t)
                    for tb in range(4):
                        xn = xp.tile([128, D], BF16, tag="xn")
                        if tb % 2 == 0:
                            nc.scalar.activation(xn[:], xts[tb][:], AF.Identity,
                                                 bias=negmrs[tb], scale=rstds[tb])
                        else:
                            nc.vector.tensor_scalar(
                                out=xn[:], in0=xts[tb][:],
                                scalar1=means[tb], scalar2=rstds[tb],
                                op0=ALU.subtract, op1=ALU.mult)
                        for ch in range(2):
                            pt = xps.tile([128, 512], BF16, tag="xtp")
                            for c in range(4):
                                cc = ch * 4 + c
                                nc.tensor.matmul(pt[:, c * 128:(c + 1) * 128],
                                                 xn[:, cc * 128:(cc + 1) * 128],
                                                 ident_r[:], is_transpose=True,
                                                 start=True, stop=True)
                            dst = xnT[:, ch * 4:(ch + 1) * 4, tb * 128:(tb + 1) * 128]
                            src = pt[:].rearrange("p (c f) -> p c f", f=128)
                            if ch == 0:
                                nc.scalar.copy(dst, src)
                            else:
                                nc.vector.tensor_copy(dst, src)
                    for m in range(2):
                        pq = pps.tile([128, 512], F32, tag="pq")
                        for c in range(8):
                            nc.tensor.matmul(pq[:], wq_r[:, c, m * 128:(m + 1) * 128],
                                             xnT[:, c, :],
                                             start=(c == 0), stop=(c == 7))
                        nc.vector.tensor_copy(
                            qT2[:, m, ic * 512:ic * 512 + 512], pq[:])
                    pkv = pps.tile([128, 512], F32, tag="pkv")
                    for c in range(8):
                        nc.tensor.matmul(pkv[:], wkv_r[:, c, :], xnT[:, c, :],
                                         start=(c == 0), stop=(c == 7))
                    nc.scalar.copy(kT[0:64, ic * 512:ic * 512 + 512], pkv[0:64, :])
                    nc.scalar.copy(kT[64:128, ic * 512:ic * 512 + 512], pkv[0:64, :])
                    vt = vtp.tile([64, 512], F32, tag="vt")
                    nc.scalar.copy(vt[:], pkv[64:128, :])
                    for tb in range(4):
                        pv = xps.tile([128, 512], F32, tag="xtp")
                        nc.tensor.matmul(pv[:, 0:64], vt[:, tb * 128:(tb + 1) * 128],
                                         ident0[0:64, 0:64], is_transpose=True,
                                         start=True, stop=True)
                        jb = ic * 4 + tb
                        nc.vector.tensor_copy(vext[:, jb * 65:jb * 65 + 64], pv[:, 0:64])

            # ------------- attention + interleaved out-proj/RS/LN -------------
            with tc.tile_pool(name="eb", bufs=6) as ebp, \
                 tc.tile_pool(name="aw", bufs=4) as awp, \
                 tc.tile_pool(name="et", bufs=3) as etp, \
                 tc.tile_pool(name="nrm", bufs=2) as nrm, \
                 tc.tile_pool(name="ysb", bufs=3) as yp, \
                 tc.tile_pool(name="fst", bufs=2) as fs, \
                 tc.tile_pool(name="aps", bufs=2, space="PSUM") as aps, \
                 tc.tile_pool(name="pops", bufs=2, space="PSUM") as pops:

                def pair_block(iq, m):
                    po2 = pops.tile([128, 1024], F32, tag="po", name=f"po{iq}{m}")
                    poA = po2[0:65, 0:512]
                    poB = po2[0:65, 512:1024]
                    q0 = iq * 512
                    aws = {}

                    def emit_sim(jb):
                        eb = ebp.tile([128, 1024], BF16, tag="eb", name=f"eb{jb}")
                        nc.sync.dma_start(
                            out=eb[:].rearrange("p (h f) -> p h f", h=2),
                            in_=expb[2 * m:2 * m + 2, jb * 128:(jb + 1) * 128,
                                     q0:q0 + 512].rearrange("h p f -> p h f"))
                        ps = aps.tile([128, 1024], F32, tag="ps", name=f"ps{jb}")
                        nc.tensor.matmul(ps[:, 0:512],
                                         kT[0:64, jb * 128:(jb + 1) * 128],
                                         qT2[0:64, m, q0:q0 + 512],
                                         start=True, stop=True)
                        nc.tensor.matmul(ps[:, 512:1024],
                                         kT[64:128, jb * 128:(jb + 1) * 128],
                                         qT2[64:128, m, q0:q0 + 512],
                                         start=True, stop=True)
                        et = etp.tile([128, 1024], BF16, tag="et", name=f"et{jb}")
                        nc.scalar.activation(et[:], ps[:], AF.Exp)
                        aw = awp.tile([128, 1024], BF16, tag="aw", name=f"aw{jb}")
                        nc.vector.tensor_mul(aw[:], et[:], eb[:])
                        aws[jb] = aw

                    def emit_av(jb):
                        aw = aws.pop(jb)
                        nc.tensor.matmul(poA, vext[:, jb * 65:jb * 65 + 65],
                                         aw[:, 0:512],
                                         start=(jb == 0), stop=(jb == 18))
                        nc.tensor.matmul(poB, vext[:, jb * 65:jb * 65 + 65],
                                         aw[:, 512:1024],
                                         start=(jb == 0), stop=(jb == 18))

                    for jb in range(19):
                        emit_sim(jb)
                        if jb >= 2:
                            emit_av(jb - 2)
                    emit_av(17)
                    emit_av(18)

                    for hh, po in ((0, poA), (1, poB)):
                        rec = nrm.tile([1, 512], F32, tag="rec", name=f"rec{hh}")
                        nc.vector.reciprocal(rec[:], po[64:65, :])
                        rbc = nrm.tile([64, 512], F32, tag="rbc", name=f"rbc{hh}")
                        nc.gpsimd.partition_broadcast(rbc[:], rec[:])
                        nc.vector.tensor_mul(
                            aoT[m][hh * 64:hh * 64 + 64, q0:q0 + 512],
                            po[0:64, :], rbc[:])

                def out_block(ib):
                    y = yp.tile([128, 1024], BF16, tag="y")
                    py = aps.tile([128, 1024], F32, tag="ps", name=f"py{ib}")
                    for ec in range(2):
                        for c in range(2):
                            nc.tensor.matmul(py[:, ec * 512:(ec + 1) * 512],
                                             aoT[c][:, ib * 128:(ib + 1) * 128],
                                             wout_r[:, c, ec * 512:(ec + 1) * 512],
                                             start=(c == 0), stop=(c == 1))
                    nc.vector.tensor_copy(y[:, 0:512], py[:, 0:512])
                    nc.scalar.copy(y[:, 512:1024], py[:, 512:1024])
                    ch = ib // 4
                    if ch < 3:
                        nc.sync.dma_start(
                            out=rs_in[ch][(ib % 4) * 128:(ib % 4 + 1) * 128, :],
                            in_=y[:])
                    else:
                        h = (ib % 4) // 2
                        nc.sync.dma_start(
                            out=rs_in3[h][(ib % 2) * 128:(ib % 2 + 1) * 128, :],
                            in_=y[:])

                def issue_rs(tin, tout):
                    nc.gpsimd.collective_compute(
                        "ReduceScatter", mybir.AluOpType.add,
                        replica_groups=[[0, 1, 2, 3], [4, 5, 6, 7]],
                        ins=[tin[:]], outs=[tout[:]])

                def final_ln(src, rows, p=128):
                    ft = yp.tile([p, 1024], F32, tag="ft")
                    nc.gpsimd.dma_start(out=ft[:], in_=src[:])
                    rstds, means, _ = _ln_stats(nc, fs, [ft[:]], D, eps_t, p=p)
                    fn = yp.tile([p, 1024], F32, tag="fn")
                    nc.vector.tensor_scalar(
                        out=fn[:], in0=ft[:], scalar1=means[0], scalar2=rstds[0],
                        op0=ALU.subtract, op1=ALU.mult)
                    nc.vector.tensor_mul(fn[:], fn[:], gamma_bc[0:p, :])
                    nc.gpsimd.dma_start(
                        out=out_ext[rows[0]:rows[1], :], in_=fn[:])

                for iq in range(4):
                    for m in range(2):
                        pair_block(iq, m)
                    if iq >= 1:
                        final_ln(rs_out[iq - 1], (128 * (iq - 1), 128 * iq))
                    for ibl in range(4):
                        out_block(iq * 4 + ibl)
                        if iq == 3 and ibl % 2 == 1:
                            issue_rs(rs_in3[ibl // 2], rs_out3[ibl // 2])
                    if iq < 3:
                        issue_rs(rs_in[iq], rs_out[iq])
                final_ln(rs_out3[0], (384, 448), p=64)
                final_ln(rs_out3[1], (448, 512), p=64)

    nc.compile()
    return nc


def _prep(inputs):
    x = np.asarray(inputs["x"], dtype=np.float32)
    context = np.asarray(inputs["context"], dtype=np.float32)
    mask = np.asarray(inputs["mask"])
    ab = np.asarray(inputs["attn_bias"], dtype=np.float32)
    norm_gamma = np.asarray(inputs["norm_gamma"], dtype=np.float32)
    null_kv = np.asarray(inputs["null_kv"], dtype=np.float32)
    Wq = np.asarray(inputs["Wq"], dtype=np.float32)
    Wkv = np.asarray(inputs["Wkv"], dtype=np.float32)
    ctx_ln_w = np.asarray(inputs["ctx_ln_w"], dtype=np.float32)
    ctx_ln_b = np.asarray(inputs["ctx_ln_b"], dtype=np.float32)
    Wctx = np.asarray(inputs["Wctx"], dtype=np.float32)
    bctx = np.asarray(inputs["bctx"], dtype=np.float32)
    Wout = np.asarray(inputs["Wout"], dtype=np.float32)
    out_gamma = np.asarray(inputs["out_gamma"], dtype=np.float32)

    scale = DH ** -0.5
    wq_f = (norm_gamma[:, None] * Wq) * scale            # (D, H*DH)
    wkv_f = np.ascontiguousarray(norm_gamma[:, None] * Wkv)
    wctx_f = np.ascontiguousarray(ctx_ln_w[:, None] * Wctx)
    bctx2 = np.ascontiguousarray((ctx_ln_b @ Wctx + bctx)[None, :])
    outg = np.ascontiguousarray(out_gamma[None, :])
    nullk = np.ascontiguousarray(np.tile(null_kv[0][:, None], (2, 1)))  # [128,1]
    nullv = np.ascontiguousarray(null_kv[1][None, :])

    # J permute [self | ctx | null], transpose j-major, exponentiate
    bp = np.concatenate([ab[:, :, C + 1:], ab[:, :, :C + 1]], axis=2)
    ebT = np.exp(np.ascontiguousarray(bp.transpose(0, 2, 1)))  # (H, J, N) f32
    mvec = np.empty((B, J), dtype=np.float32)
    mvec[:, :N] = mask[:, C:]
    mvec[:, N] = 1.0                       # ctx[0]: the left-pad True
    mvec[:, N + 1:N + C] = mask[:, :C - 1]  # ctx[c] <- mask[c-1]
    mvec[:, N + C] = mask[:, C - 1]         # null <- mask[255]

    in_maps = []
    for core in range(8):
        b, g = core // 4, core % 4
        eb = ebT[HPC * g:HPC * g + HPC] * mvec[b][None, :, None]
        ebp = np.zeros((HPC, JP, N), dtype=ml_dtypes.bfloat16)
        ebp[:, :J, :] = eb.astype(ml_dtypes.bfloat16)
        in_maps.append({
            "expb": ebp,
            "x": np.ascontiguousarray(x[b]),
            "ctxt": np.ascontiguousarray(context[b]),
            "nullk": nullk,
            "nullv": nullv,
            "wq": np.ascontiguousarray(wq_f[:, 256 * g:256 * (g + 1)]),
            "wkv": wkv_f,
            "wctx": wctx_f,
            "bctx2": bctx2,
            "wout": np.ascontiguousarray(Wout[256 * g:256 * (g + 1), :]),
            "outg": outg,
        })
    return in_maps


def kernel(**inputs) -> np.ndarray:
    if "nc" not in _cache:
        _cache["nc"] = build()
    nc = _cache["nc"]
    in_maps = _prep(inputs)
    res = run_bass_kernel_spmd(nc, in_maps, core_ids=list(range(8))).results
    out = np.empty((B, N, D), dtype=np.float32)
    for core in range(8):
        b, r = core // 4, core % 4
        o = res[core]["out"]
        for ch in range(3):
            out[b, 512 * ch + 128 * r:512 * ch + 128 * (r + 1), :] = \
                o[ch * 128:(ch + 1) * 128]
        out[b, 1536 + 64 * r:1536 + 64 * (r + 1), :] = o[384:448]
        out[b, 1792 + 64 * r:1792 + 64 * (r + 1), :] = o[448:512]
    return out


# revision 18
# speedup vs baseline: 1.1562x; 1.1562x over previous
"""Trainium2 distributed kernel for the multi-query sparse-attention block.

Sharding: 8 cores = 2 batches x 4 head-groups (4 heads each).
J (key/value axis) is host-permuted to [self(2048) | ctx(256) | null(1) | pad(127)]
and the attention bias arrives pre-transposed (j-major), mask-folded and
pre-exponentiated in bf16:  attn_weight = exp(q.k) * expb.

Attention processes HEAD PAIRS: head A lives on SBUF partitions 0-63, head B
on 64-127 (kT/qT duplicated/stacked), so the two qk matmuls run CONCURRENTLY
on the PE's two 64-row tiles (tile_position (0,0) / (64,0)).  The two sim
tiles land in adjacent PSUM banks and are exponentiated by a single 1024-wide
scalar-engine instruction (the scalar engine exp stream is the pacing
resource).  Softmax runs without max-subtraction; denominators come from a
ones-column appended to V and are reciprocated on the vector engine.  All
layernorm rstds use exp(-0.5*ln(var+eps)) so the scalar engine never switches
activation tables.  Output projection partials are ReduceScattered over each
4-core batch group; final layernorms are pipelined into the attention loop.
"""

import sys

sys.path.insert(0, "/opt/trn_rl_repo")

import numpy as np
import ml_dtypes

import concourse.bass as bass
import concourse.mybir as mybir
import concourse.tile as tile
from concourse import bacc
from concourse.bass_utils import run_bass_kernel_spmd
from concourse.masks import make_identity

F32 = mybir.dt.float32
F32R = mybir.dt.float32r
BF16 = mybir.dt.bfloat16
AF = mybir.ActivationFunctionType
ALU = mybir.AluOpType

B, N, D = 2, 2048, 1024
H, DH = 16, 64
C, CD = 256, 512
J = C + 1 + N          # 2305
JP = 19 * 128          # 2432 padded
HPC = 4                # heads per core
EPS = 1e-5

_cache = {}


I32 = mybir.dt.int32
QMAGIC = 0x5F3759DF


def _ln_stats(nc, pool, srcs, d, eps_ap, p=128):
    """Batched LN stats for k [p, d] tiles.  rstd = rsqrt(var+eps) computed
    entirely on the vector engine (Quake bit-trick seed + 2 Newton steps) so
    the scalar engine never loads the sqrt/ln activation tables.
    Returns (rstd_aps, mean_aps, negmr_aps)."""
    k = len(srcs)
    ns = d // 512
    mv = pool.tile([p, k, 2], F32, tag="lnmv")
    for j, s_ap in enumerate(srcs):
        stats = pool.tile([p, ns, 6], F32, tag="lnst")
        r = s_ap.rearrange("p (n f) -> p n f", f=512)
        for s in range(ns):
            nc.vector.bn_stats(out=stats[:, s, :], in_=r[:, s, :])
        nc.vector.bn_aggr(out=mv[:, j, :], in_=stats[:, :, :])
    mvr = mv[:].rearrange("p k two -> p (k two)")
    t = pool.tile([p, k], F32, tag="lnt")
    nc.vector.tensor_scalar_add(t[:], mvr[:, 1::2], EPS)
    yi = pool.tile([p, k], I32, tag="lnyi")
    nc.vector.tensor_scalar(out=yi[:], in0=t[:].bitcast(I32),
                            scalar1=1, scalar2=None,
                            op0=ALU.arith_shift_right)
    nc.vector.tensor_scalar(out=yi[:], in0=yi[:], scalar1=-1, scalar2=QMAGIC,
                            op0=ALU.mult, op1=ALU.add)
    y1 = pool.tile([p, k], F32, tag="lny1")
    rstd = pool.tile([p, k], F32, tag="lnrs")
    u = pool.tile([p, k], F32, tag="lnu")
    cur = yi[:].bitcast(F32)
    for it, dst in ((0, y1), (1, rstd)):
        nc.vector.tensor_mul(u[:], t[:], cur)
        nc.vector.tensor_mul(u[:], u[:], cur)
        nc.vector.tensor_scalar(out=u[:], in0=u[:], scalar1=-0.5, scalar2=1.5,
                                op0=ALU.mult, op1=ALU.add)
        nc.vector.tensor_mul(dst[:], cur, u[:])
        cur = dst[:]
    negmr = pool.tile([p, k], F32, tag="lnnm")
    nc.vector.scalar_tensor_tensor(
        out=negmr[:], in0=mvr[:, 0::2], scalar=-1.0, in1=rstd[:],
        op0=ALU.mult, op1=ALU.mult)
    return ([rstd[:, j:j + 1] for j in range(k)],
            [mvr[:, 2 * j:2 * j + 1] for j in range(k)],
            [negmr[:, j:j + 1] for j in range(k)])


def build():
    nc = bacc.Bacc("TRN2", target_bir_lowering=False, debug=False, num_devices=8)

    expb = nc.declare_dram_parameter("expb", [HPC, JP, N], BF16, isOutput=False)
    x_in = nc.declare_dram_parameter("x", [N, D], F32, isOutput=False)
    ctx_in = nc.declare_dram_parameter("ctxt", [C, CD], F32, isOutput=False)
    nullk = nc.declare_dram_parameter("nullk", [128, 1], F32, isOutput=False)
    nullv = nc.declare_dram_parameter("nullv", [1, DH], F32, isOutput=False)
    wq_in = nc.declare_dram_parameter("wq", [D, 256], F32, isOutput=False)
    wkv_in = nc.declare_dram_parameter("wkv", [D, 128], F32, isOutput=False)
    wctx_in = nc.declare_dram_parameter("wctx", [CD, 128], F32, isOutput=False)
    bctx_in = nc.declare_dram_parameter("bctx2", [1, 128], F32, isOutput=False)
    wout_in = nc.declare_dram_parameter("wout", [256, D], F32, isOutput=False)
    outg_in = nc.declare_dram_parameter("outg", [1, D], F32, isOutput=False)
    out_ext = nc.declare_dram_parameter("out", [N // 4, D], F32, isOutput=True)

    rs_in = [nc.dram_tensor(f"rs_in{c}", [512, D], BF16) for c in range(3)]
    rs_out = [nc.dram_tensor(f"rs_out{c}", [128, D], BF16) for c in range(3)]
    # last chunk split in two for a shorter exposed tail
    rs_in3 = [nc.dram_tensor(f"rs_in3{c}", [256, D], BF16) for c in range(2)]
    rs_out3 = [nc.dram_tensor(f"rs_out3{c}", [64, D], BF16) for c in range(2)]

    with tile.TileContext(nc) as tc:
        with tc.tile_pool(name="persist", bufs=1) as pp:
            wq_r = pp.tile([128, 8, 256], BF16)
            wkv_r = pp.tile([128, 8, 128], BF16)
            wctx_r = pp.tile([128, 4, 128], BF16)
            wout_r = pp.tile([128, 2, 1024], F32R)
            bctx_r = pp.tile([1, 128], BF16)
            ones_r = pp.tile([1, 1024], BF16)
            ident0 = pp.tile([128, 128], F32)
            ident_r = pp.tile([128, 128], BF16)
            gamma_bc = pp.tile([128, 1024], F32)
            qT2 = pp.tile([128, 2, N], BF16)     # [pairstack, m, n]
            kT = pp.tile([128, JP], BF16)        # both halves hold same kT
            vext = pp.tile([128, 19 * 65], BF16)
            aoT0 = pp.tile([128, N], F32R)
            aoT1 = pp.tile([128, N], F32R)
            aoT = [aoT0, aoT1]

            nc.gpsimd.dma_start(out=wq_r[:], in_=wq_in.rearrange("(c p) f -> p c f", p=128))
            nc.gpsimd.dma_start(out=wkv_r[:], in_=wkv_in.rearrange("(c p) f -> p c f", p=128))
            nc.gpsimd.dma_start(out=wctx_r[:], in_=wctx_in.rearrange("(c p) f -> p c f", p=128))
            nc.gpsimd.dma_start(out=wout_r[:], in_=wout_in.rearrange("(c p) f -> p c f", p=128))
            nc.gpsimd.dma_start(out=bctx_r[:], in_=bctx_in[:])
            nc.gpsimd.dma_start(out=kT[:, 2304:2305], in_=nullk[:])

            eps_t = pp.tile([128, 1], F32)
            nc.gpsimd.memset(eps_t[:], EPS)
            nc.vector.memset(kT[:, 2305:2432], 0.0)
            nc.vector.memset(vext[:, 18 * 65:18 * 65 + 64], 0.0)
            nc.gpsimd.dma_start(out=vext[0:1, 18 * 65:18 * 65 + 64], in_=nullv[:])
            for jb in range(19):
                nc.vector.memset(vext[:, jb * 65 + 64:jb * 65 + 65], 1.0)

            o1 = pp.tile([1, 1024], F32)
            nc.vector.memset(o1[:], 1.0)
            nc.scalar.copy(ones_r[:], o1[:])
            make_identity(nc, ident0[:])
            nc.scalar.copy(ident_r[:], ident0[:])

            og_sb = pp.tile([1, 1024], F32)
            nc.sync.dma_start(out=og_sb[:], in_=outg_in[:])
            nc.gpsimd.partition_broadcast(gamma_bc[:], og_sb[:])

            # ---------------- context tokens -> kT/vext ----------------
            # (emitted first: its compute overlaps the large x DMAs)
            with tc.tile_pool(name="cwork", bufs=2) as cw, \
                 tc.tile_pool(name="cstat", bufs=2) as cs, \
                 tc.tile_pool(name="cps", bufs=2, space="PSUM") as cps:
                cnT = pp.tile([128, 4, 256], BF16)
                cts = []
                for t in range(2):
                    ct = cw.tile([128, CD], F32, tag="ct")
                    nc.sync.dma_start(out=ct[:], in_=ctx_in[t * 128:(t + 1) * 128, :])
                    cts.append(ct)
                rstds, means, negmrs = _ln_stats(
                    nc, cs, [c[:] for c in cts], CD, eps_t)
                for t in range(2):
                    cn = cw.tile([128, CD], BF16, tag="cn")
                    nc.scalar.activation(cn[:], cts[t][:], AF.Identity,
                                         bias=negmrs[t], scale=rstds[t])
                    for c in range(4):
                        pt = cps.tile([128, 128], BF16, tag="ctp")
                        nc.tensor.matmul(pt[:], cn[:, c * 128:(c + 1) * 128],
                                         ident_r[:], is_transpose=True,
                                         start=True, stop=True)
                        nc.scalar.copy(cnT[:, c, t * 128:(t + 1) * 128],
                                       pt[:])
                pck = cps.tile([64, 256], F32, tag="ck")
                for c in range(4):
                    nc.tensor.matmul(pck[:], wctx_r[:, c, 0:64], cnT[:, c, :],
                                     start=(c == 0), stop=False)
                nc.tensor.matmul(pck[:], bctx_r[:, 0:64], ones_r[:, 0:256],
                                 start=False, stop=True)
                nc.scalar.copy(kT[0:64, 2048:2304], pck[:])
                nc.scalar.copy(kT[64:128, 2048:2304], pck[:])
                for t in range(2):
                    pcv = cps.tile([128, 64], F32, tag="cv")
                    for c in range(4):
                        nc.tensor.matmul(pcv[:], cnT[:, c, t * 128:(t + 1) * 128],
                                         wctx_r[:, c, 64:128],
                                         start=(c == 0), stop=False)
                    nc.tensor.matmul(pcv[:], ones_r[:, 0:128], bctx_r[:, 64:128],
                                     start=False, stop=True)
                    nc.vector.tensor_copy(vext[:, (16 + t) * 65:(16 + t) * 65 + 64],
                                          pcv[:])

            # ---------------- x: LN + transpose + projections ----------------
            with tc.tile_pool(name="xt", bufs=16) as xp, \
                 tc.tile_pool(name="xst", bufs=2) as xs, \
                 tc.tile_pool(name="xnt", bufs=2) as xnp, \
                 tc.tile_pool(name="vtmp", bufs=2) as vtp, \
                 tc.tile_pool(name="xps", bufs=3, space="PSUM") as xps, \
                 tc.tile_pool(name="pps", bufs=2, space="PSUM") as pps:
                for ic in range(4):
                    xnT = xnp.tile([128, 8, 512], BF16, tag="xnT")
                    xts = []
                    for tb in range(4):
                        i0 = ic * 512 + tb * 128
                        xt = xp.tile([128, D], F32, tag="xt")
                        nc.sync.dma_start(out=xt[:], in_=x_in[i0:i0 + 128, :])
                        xts.append(xt)
                    rstds, means, negmrs = _ln_stats(
                        nc, xs, [t[:] for t in xts], D, eps_t)
                    for tb in range(4):
                        xn = xp.tile([128, D], BF16, tag="xn")
                        if tb % 2 == 0:
                            nc.scalar.activation(xn[:], xts[tb][:], AF.Identity,
                                                 bias=negmrs[tb], scale=rstds[tb])
                        else:
                            nc.vector.tensor_scalar(
                                out=xn[:], in0=xts[tb][:],
                                scalar1=means[tb], scalar2=rstds[tb],
                                op0=ALU.subtract, op1=ALU.mult)
                        for ch in range(2):
                            pt = xps.tile([128, 512], BF16, tag="xtp")
                            for c in range(4):
                                cc = ch * 4 + c
                                nc.tensor.matmul(pt[:, c * 128:(c + 1) * 128],
                                                 xn[:, cc * 128:(cc + 1) * 128],
                                                 ident_r[:], is_transpose=True,
                                                 start=True, stop=True)
                            dst = xnT[:, ch * 4:(ch + 1) * 4, tb * 128:(tb + 1) * 128]
                            src = pt[:].rearrange("p (c f) -> p c f", f=128)
                            if ch == 0:
                                nc.scalar.copy(dst, src)
                            else:
                                nc.vector.tensor_copy(dst, src)
                    for m in range(2):
                        pq = pps.tile([128, 512], F32, tag="pq")
                        for c in range(8):
                            nc.tensor.matmul(pq[:], wq_r[:, c, m * 128:(m + 1) * 128],
                                             xnT[:, c, :],
                                             start=(c == 0), stop=(c == 7))
                        nc.vector.tensor_copy(
                            qT2[:, m, ic * 512:ic * 512 + 512], pq[:])
                    pkv = pps.tile([128, 512], F32, tag="pkv")
                    for c in range(8):
                        nc.tensor.matmul(pkv[:], wkv_r[:, c, :], xnT[:, c, :],
                                         start=(c == 0), stop=(c == 7))
                    nc.scalar.copy(kT[0:64, ic * 512:ic * 512 + 512], pkv[0:64, :])
                    nc.scalar.copy(kT[64:128, ic * 512:ic * 512 + 512], pkv[0:64, :])
                    vt = vtp.tile([64, 512], F32, tag="vt")
                    nc.scalar.copy(vt[:], pkv[64:128, :])
                    for tb in range(4):
                        pv = xps.tile([128, 512], F32, tag="xtp")
                        nc.tensor.matmul(pv[:, 0:64], vt[:, tb * 128:(tb + 1) * 128],
                                         ident0[0:64, 0:64], is_transpose=True,
                                         start=True, stop=True)
                        jb = ic * 4 + tb
                        nc.vector.tensor_copy(vext[:, jb * 65:jb * 65 + 64], pv[:, 0:64])

            # ------------- attention + interleaved out-proj/RS/LN -------------
            with tc.tile_pool(name="eb", bufs=6) as ebp, \
                 tc.tile_pool(name="aw", bufs=4) as awp, \
                 tc.tile_pool(name="et", bufs=3) as etp, \
                 tc.tile_pool(name="nrm", bufs=2) as nrm, \
                 tc.tile_pool(name="ysb", bufs=3) as yp, \
                 tc.tile_pool(name="fst", bufs=2) as fs, \
                 tc.tile_pool(name="aps", bufs=2, space="PSUM") as aps, \
                 tc.tile_pool(name="pops", bufs=2, space="PSUM") as pops:

                def pair_block(iq, m, deferred=()):
                    dq = list(deferred)
                    po2 = pops.tile([128, 1024], F32, tag="po", name=f"po{iq}{m}")
                    poA = po2[0:65, 0:512]
                    poB = po2[0:65, 512:1024]
                    q0 = iq * 512
                    aws = {}

                    def emit_sim(jb):
                        eb = ebp.tile([128, 1024], BF16, tag="eb", name=f"eb{jb}")
                        nc.sync.dma_start(
                            out=eb[:].rearrange("p (h f) -> p h f", h=2),
                            in_=expb[2 * m:2 * m + 2, jb * 128:(jb + 1) * 128,
                                     q0:q0 + 512].rearrange("h p f -> p h f"))
                        ps = aps.tile([128, 1024], F32, tag="ps", name=f"ps{jb}")
                        nc.tensor.matmul(ps[:, 0:512],
                                         kT[0:64, jb * 128:(jb + 1) * 128],
                                         qT2[0:64, m, q0:q0 + 512],
                                         start=True, stop=True)
                        nc.tensor.matmul(ps[:, 512:1024],
                                         kT[64:128, jb * 128:(jb + 1) * 128],
                                         qT2[64:128, m, q0:q0 + 512],
                                         start=True, stop=True)
                        et = etp.tile([128, 1024], BF16, tag="et", name=f"et{jb}")
                        nc.scalar.activation(et[:], ps[:], AF.Exp)
                        aw = awp.tile([128, 1024], BF16, tag="aw", name=f"aw{jb}")
                        nc.vector.tensor_mul(aw[:], et[:], eb[:])
                        aws[jb] = aw

                    def emit_av(jb):
                        aw = aws.pop(jb)
                        nc.tensor.matmul(poA, vext[:, jb * 65:jb * 65 + 65],
                                         aw[:, 0:512],
                                         start=(jb == 0), stop=(jb == 18))
                        nc.tensor.matmul(poB, vext[:, jb * 65:jb * 65 + 65],
                                         aw[:, 512:1024],
                                         start=(jb == 0), stop=(jb == 18))

                    for jb in range(19):
                        emit_sim(jb)
                        if jb >= 2:
                            emit_av(jb - 2)
                        if jb >= 6 and jb % 3 == 0 and dq:
                            dq.pop(0)()
                    emit_av(17)
                    emit_av(18)
                    while dq:
                        dq.pop(0)()

                    for hh, po in ((0, poA), (1, poB)):
                        rec = nrm.tile([1, 512], F32, tag="rec", name=f"rec{hh}")
                        nc.vector.reciprocal(rec[:], po[64:65, :])
                        rbc = nrm.tile([64, 512], F32, tag="rbc", name=f"rbc{hh}")
                        nc.gpsimd.partition_broadcast(rbc[:], rec[:])
                        nc.vector.tensor_mul(
                            aoT[m][hh * 64:hh * 64 + 64, q0:q0 + 512],
                            po[0:64, :], rbc[:])

                def out_block(ib):
                    y = yp.tile([128, 1024], BF16, tag="y")
                    py = aps.tile([128, 1024], F32, tag="ps", name=f"py{ib}")
                    for ec in range(2):
                        for c in range(2):
                            nc.tensor.matmul(py[:, ec * 512:(ec + 1) * 512],
                                             aoT[c][:, ib * 128:(ib + 1) * 128],
                                             wout_r[:, c, ec * 512:(ec + 1) * 512],
                                             start=(c == 0), stop=(c == 1))
                    nc.vector.tensor_copy(y[:, 0:512], py[:, 0:512])
                    nc.scalar.copy(y[:, 512:1024], py[:, 512:1024])
                    ch = ib // 4
                    if ch < 3:
                        nc.gpsimd.dma_start(
                            out=rs_in[ch][(ib % 4) * 128:(ib % 4 + 1) * 128, :],
                            in_=y[:])
                    else:
                        h = (ib % 4) // 2
                        nc.gpsimd.dma_start(
                            out=rs_in3[h][(ib % 2) * 128:(ib % 2 + 1) * 128, :],
                            in_=y[:])

                def issue_rs(tin, tout):
                    nc.gpsimd.collective_compute(
                        "ReduceScatter", mybir.AluOpType.add,
                        replica_groups=[[0, 1, 2, 3], [4, 5, 6, 7]],
                        ins=[tin[:]], outs=[tout[:]])

                def final_ln(src, rows, p=128):
                    ft = yp.tile([p, 1024], F32, tag="ft")
                    nc.gpsimd.dma_start(out=ft[:], in_=src[:])
                    rstds, means, _ = _ln_stats(nc, fs, [ft[:]], D, eps_t, p=p)
                    fn = yp.tile([p, 1024], F32, tag="fn")
                    nc.vector.tensor_scalar(
                        out=fn[:], in0=ft[:], scalar1=means[0], scalar2=rstds[0],
                        op0=ALU.subtract, op1=ALU.mult)
                    nc.vector.tensor_mul(fn[:], fn[:], gamma_bc[0:p, :])
                    nc.gpsimd.dma_start(
                        out=out_ext[rows[0]:rows[1], :], in_=fn[:])

                for iq in range(4):
                    d0, d1 = [], []
                    if iq >= 1:
                        pq = iq - 1
                        d0 = [(lambda ibl=ibl, pq=pq: out_block(4 * pq + ibl))
                              for ibl in range(4)]
                        d0.append(lambda pq=pq: issue_rs(rs_in[pq], rs_out[pq]))
                    if iq >= 2:
                        ch = iq - 2
                        d1 = [lambda ch=ch: final_ln(
                            rs_out[ch], (128 * ch, 128 * (ch + 1)))]
                    pair_block(iq, 0, d0)
                    pair_block(iq, 1, d1)
                for ibl in range(4):
                    out_block(12 + ibl)
                    if ibl % 2 == 1:
                        issue_rs(rs_in3[ibl // 2], rs_out3[ibl // 2])
                final_ln(rs_out[2], (256, 384))
                final_ln(rs_out3[0], (384, 448), p=64)
                final_ln(rs_out3[1], (448, 512), p=64)

    nc.compile()
    return nc


def _prep(inputs):
    x = np.asarray(inputs["x"], dtype=np.float32)
    context = np.asarray(inputs["context"], dtype=np.float32)
    mask = np.asarray(inputs["mask"])
    ab = np.asarray(inputs["attn_bias"], dtype=np.float32)
    norm_gamma = np.asarray(inputs["norm_gamma"], dtype=np.float32)
    null_kv = np.asarray(inputs["null_kv"], dtype=np.float32)
    Wq = np.asarray(inputs["Wq"], dtype=np.float32)
    Wkv = np.asarray(inputs["Wkv"], dtype=np.float32)
    ctx_ln_w = np.asarray(inputs["ctx_ln_w"], dtype=np.float32)
    ctx_ln_b = np.asarray(inputs["ctx_ln_b"], dtype=np.float32)
    Wctx = np.asarray(inputs["Wctx"], dtype=np.float32)
    bctx = np.asarray(inputs["bctx"], dtype=np.float32)
    Wout = np.asarray(inputs["Wout"], dtype=np.float32)
    out_gamma = np.asarray(inputs["out_gamma"], dtype=np.float32)

    scale = DH ** -0.5
    wq_f = (norm_gamma[:, None] * Wq) * scale            # (D, H*DH)
    wkv_f = np.ascontiguousarray(norm_gamma[:, None] * Wkv)
    wctx_f = np.ascontiguousarray(ctx_ln_w[:, None] * Wctx)
    bctx2 = np.ascontiguousarray((ctx_ln_b @ Wctx + bctx)[None, :])
    outg = np.ascontiguousarray(out_gamma[None, :])
    nullk = np.ascontiguousarray(np.tile(null_kv[0][:, None], (2, 1)))  # [128,1]
    nullv = np.ascontiguousarray(null_kv[1][None, :])

    # J permute [self | ctx | null], transpose j-major, exponentiate
    bp = np.concatenate([ab[:, :, C + 1:], ab[:, :, :C + 1]], axis=2)
    ebT = np.exp(np.ascontiguousarray(bp.transpose(0, 2, 1)))  # (H, J, N) f32
    mvec = np.empty((B, J), dtype=np.float32)
    mvec[:, :N] = mask[:, C:]
    mvec[:, N] = 1.0                       # ctx[0]: the left-pad True
    mvec[:, N + 1:N + C] = mask[:, :C - 1]  # ctx[c] <- mask[c-1]
    mvec[:, N + C] = mask[:, C - 1]         # null <- mask[255]

    in_maps = []
    for core in range(8):
        b, g = core // 4, core % 4
        eb = ebT[HPC * g:HPC * g + HPC] * mvec[b][None, :, None]
        ebp = np.zeros((HPC, JP, N), dtype=ml_dtypes.bfloat16)
        ebp[:, :J, :] = eb.astype(ml_dtypes.bfloat16)
        in_maps.append({
            "expb": ebp,
            "x": np.ascontiguousarray(x[b]),
            "ctxt": np.ascontiguousarray(context[b]),
            "nullk": nullk,
            "nullv": nullv,
            "wq": np.ascontiguousarray(wq_f[:, 256 * g:256 * (g + 1)]),
            "wkv": wkv_f,
            "wctx": wctx_f,
            "bctx2": bctx2,
            "wout": np.ascontiguousarray(Wout[256 * g:256 * (g + 1), :]),
            "outg": outg,
        })
    return in_maps


def kernel(**inputs) -> np.ndarray:
    if "nc" not in _cache:
        _cache["nc"] = build()
    nc = _cache["nc"]
    in_maps = _prep(inputs)
    res = run_bass_kernel_spmd(nc, in_maps, core_ids=list(range(8))).results
    out = np.empty((B, N, D), dtype=np.float32)
    for core in range(8):
        b, r = core // 4, core % 4
        o = res[core]["out"]
        for ch in range(3):
            out[b, 512 * ch + 128 * r:512 * ch + 128 * (r + 1), :] = \
                o[ch * 128:(ch + 1) * 128]
        out[b, 1536 + 64 * r:1536 + 64 * (r + 1), :] = o[384:448]
        out[b, 1792 + 64 * r:1792 + 64 * (r + 1), :] = o[448:512]
    return out
